# revision 2
# baseline (speedup 1.0000x reference)
"""ConcatCritic MLP on 8 Trainium2 NeuronCores.

Computes out[a, b] = f(concat(x[a], y[b])) for a tiny 4-layer MLP
(256->128->64->8->1 with ReLU), i.e. a [1024, 1024] score matrix.

Sharding (per spec hint): x's batch dim across the 8 cores (128 rows each);
y and the weights replicated. Each core computes a [128, 1024] output block.

Dataflow per core (feature-on-partition layout):
  - Split layer 1: concat(x,y) @ W1 = x @ W1[:128] + y @ W1[128:].
    xab[f, i] = (W1a^T @ x^T)[f, i] + b1[f]   (128 x 128, fp32)
    ybT[f, j] = (W1b^T @ y^T)[f, j]           (128 x 1024, bf16)
  - h1_i = relu(ybT + xab[:, i]) -- one tensor_scalar per i (bf16); even i
    on the DVE, odd i on GpSimd (SBUF-only op, so GpSimd is legal and it
    offloads the DVE, which also handles PSUM-side evacuation spill).
  - L2: PE matmul, stationary W2 [128, 64]; even i -> PSUM rows 0:64
    (tile_position (0,0)), odd i -> rows 64:128 ((0,64)); one PSUM tile
    holds a pair of i's -> relu(+b2) evacuates [128, 1024] at once
    (ScalarE, with every 4th pair's second half on the DVE).
  - L3: stationary [128, 64] zero-padded 16-col strips (variant v for pair
    p = 4t+v) accumulate 4 pairs into each 64-row PSUM half; 8 pairs fill a
    dense [128, 1024] "h3pack" (16 i's) -> relu(+b3) evacuates split
    across ScalarE/DVE halves.
  - L4: stationary [128, 64] with one W4 entry per (row-block, i) strip;
    8 groups accumulate into one [128, 1024] PSUM = the core's full output
    block (+b4 on evacuation).

Pipelining: the PE is the roofline engine (~74us of matmul exec), so its
queue must never wait on an evacuation. Both L3 and L4 are DEFERRED: the
PE program order per pair-slot is [L2(p) x4][L4(g-1) at p==2][L3(p-1) x2],
so every matmul's input evacuation completed during the previous slot.
h1 production runs LOOKAHEAD pairs ahead.

All matmuls are bf16 (2 cols/cycle) in 128x64 column-tiling mode
(tile_positions (0,0)/(0,64) only) so the PE never mode-switches. PSUM
accumulation is fp32. x^T/y^T are cast to bf16 on the HOST and arrive in
dependency-ordered DMA chunks so layer 1 starts ~0.2us after launch.
PSUM budget: ps2 2x[128,1024] (4 banks) + ps3 1x (2) + pso 1x (2) = 8.
"""

import numpy as np
import ml_dtypes

import concourse.bass as bass
import concourse.bacc as bacc
import concourse.mybir as mybir
import concourse.tile as tile
from concourse.bass_utils import run_bass_kernel_spmd

BF16 = ml_dtypes.bfloat16
N_CORES = 8
B = 1024
D = 128
NI = B // N_CORES  # 128 rows of x per core
J = B              # full y batch per core
JC = 512           # matmul free-dim chunk (one PSUM bank)

# bin1 [128, 256] bf16: W1A | xT shard     (critical path: layer-1 x side)
A_W1A = 0
A_XT = A_W1A + D
A_TOT = A_XT + NI
# bin2 [128, 1152] bf16: W1B | yT          (layer-1 y side)
B_W1B = 0
B_YT = B_W1B + D
B_TOT = B_YT + J
# bin3 [128, 576] bf16: W2 | W3P | W4P     (needed from first L2 on)
C_W2 = 0
C_W3P = C_W2 + 64
C_W4P = C_W3P + 256
C_TOT = C_W4P + 256
# fin [128, 4] fp32: b1 | b2(x2) | b3(x16) | b4
F_B1, F_B2, F_B3, F_B4 = 0, 1, 2, 3

# h2 evacuation: pairs with (gp % EVAC_DVE_MOD) == EVAC_DVE_PHASE get their
# second column half evacuated by the DVE instead of ScalarE, keeping the
# ScalarE (whose steady load ~= the PE's) from pacing the kernel.
EVAC_DVE_MOD = 4
EVAC_DVE_PHASE = 2

LOOKAHEAD = 3
NPAIR = 64

_CACHE = {}


def _i_local_of_row(r):
    # h3pack row r -> which of the group's 16 i's it holds
    t, v, b = r // 64, (r % 64) // 16, (r % 16) // 8
    return 2 * (4 * t + v) + b


def _build_packed_weights(W3, W4):
    W3P = np.zeros((4, 128, 64), np.float32)
    for v in range(4):
        W3P[v, 0:64, 16 * v : 16 * v + 8] = W3
        W3P[v, 64:128, 16 * v + 8 : 16 * v + 16] = W3
    W4P = np.zeros((4, 128, 64), np.float32)
    for v4 in range(4):
        for r in range(128):
            c = 16 * v4 + _i_local_of_row(r)
            W4P[v4, r, c] = W4[r % 8, 0]
    return W3P, W4P


def _build_bass():
    nc = bacc.Bacc("TRN2", target_bir_lowering=False)
    f32 = mybir.dt.float32
    bf16 = mybir.dt.bfloat16

    bin1 = nc.dram_tensor("bin1", [D, A_TOT], bf16, kind="ExternalInput")
    bin2 = nc.dram_tensor("bin2", [D, B_TOT], bf16, kind="ExternalInput")
    bin3 = nc.dram_tensor("bin3", [D, C_TOT], bf16, kind="ExternalInput")
    fin_d = nc.dram_tensor("fin", [D, 4], f32, kind="ExternalInput")
    outd = nc.dram_tensor("out", [NI, J], f32, kind="ExternalOutput")

    RELU = mybir.ActivationFunctionType.Relu
    IDENT = mybir.ActivationFunctionType.Identity
    ADD = mybir.AluOpType.add
    MAX = mybir.AluOpType.max

    with tile.TileContext(nc) as tc:
        with (
            tc.tile_pool(name="const", bufs=1) as cpool,
            tc.tile_pool(name="work", bufs=10) as work,
            tc.tile_pool(name="h2p", bufs=6) as h2pool,
            tc.tile_pool(name="h3p", bufs=2) as h3pool,
            tc.tile_pool(name="ps2", bufs=2, space="PSUM") as ps2,
            tc.tile_pool(name="ps3", bufs=1, space="PSUM") as ps3,
            tc.tile_pool(name="pso", bufs=1, space="PSUM") as pso,
        ):
            hin1 = cpool.tile([D, A_TOT], bf16)
            hin2 = cpool.tile([D, B_TOT], bf16)
            hin3 = cpool.tile([D, C_TOT], bf16)
            fin = cpool.tile([D, 4], f32)
            xab = cpool.tile([D, NI], f32)
            ybT = cpool.tile([D, J], bf16)

            # dependency-ordered input DMAs (critical first)
            nc.sync.dma_start(hin1[:], bin1[:])
            nc.sync.dma_start(fin[:], fin_d[:])
            nc.sync.dma_start(hin2[:], bin2[:])
            nc.sync.dma_start(hin3[:], bin3[:])

            W2_sb = hin3[:, C_W2 : C_W2 + 64]
            b1_sb = fin[:, F_B1 : F_B1 + 1]
            b2_sb = fin[:, F_B2 : F_B2 + 1]
            b3_sb = fin[:, F_B3 : F_B3 + 1]
            b4_sb = fin[:, F_B4 : F_B4 + 1]

            # ---- precompute: xab = W1a^T @ xT + b1 ; ybT = W1b^T @ yT ----
            xa_ps = ps2.tile([D, J], mybir.dt.float32, tag="ps2")
            for c in range(2):
                nc.tensor.matmul(
                    xa_ps[64 * c : 64 * c + 64, :NI],
                    hin1[:, A_W1A + 64 * c : A_W1A + 64 * c + 64],
                    hin1[:, A_XT : A_XT + NI],
                    tile_position=(0, 64 * c),
                )
            nc.scalar.activation(xab[:], xa_ps[:, :NI], IDENT, bias=b1_sb)

            yb_ps = ps2.tile([D, J], mybir.dt.float32, tag="ps2")
            for c in range(2):
                for jc in range(2):
                    nc.tensor.matmul(
                        yb_ps[64 * c : 64 * c + 64, JC * jc : JC * jc + JC],
                        hin2[:, B_W1B + 64 * c : B_W1B + 64 * c + 64],
                        hin2[:, B_YT + JC * jc : B_YT + JC * jc + JC],
                        tile_position=(0, 64 * c),
                    )
            nc.scalar.activation(ybT[:], yb_ps[:], IDENT)

            # ---- main loop (software-pipelined emission) ----
            out_ps = pso.tile([D, J], mybir.dt.float32)
            out_sb = cpool.tile([NI, J], f32)

            def x_col(gp):
                g, p = gp // 8, gp % 8
                return 64 * (g // 4) + 16 * (g % 4) + 2 * p

            def emit_h1(gp):
                h1e = work.tile([D, J], bf16, tag="h1")
                h1o = work.tile([D, J], bf16, tag="h1")
                ie = x_col(gp)
                nc.vector.tensor_scalar(
                    h1e[:], ybT[:], xab[:, ie : ie + 1], 0.0, ADD, MAX
                )
                nc.gpsimd.tensor_scalar(
                    h1o[:], ybT[:], xab[:, ie + 1 : ie + 2], 0.0, ADD, MAX
                )
                return h1e, h1o

            def _emit_l4(g, h3pack):
                t4, v4 = g // 4, g % 4
                for jc in range(2):
                    jsl = slice(JC * jc, JC * jc + JC)
                    nc.tensor.matmul(
                        out_ps[64 * t4 : 64 * t4 + 64, jsl],
                        hin3[:, C_W4P + 64 * v4 : C_W4P + 64 * v4 + 64],
                        h3pack[:, jsl],
                        tile_position=(0, 64 * t4),
                        start=(v4 == 0),
                        stop=(v4 == 3),
                    )

            def _emit_l3(t, v, h2pack, ps3_ref):
                for jc in range(2):
                    jsl = slice(JC * jc, JC * jc + JC)
                    nc.tensor.matmul(
                        ps3_ref[64 * t : 64 * t + 64, jsl],
                        hin3[:, C_W3P + 64 * v : C_W3P + 64 * v + 64],
                        h2pack[:, jsl],
                        tile_position=(0, 64 * t),
                        start=(v == 0),
                        stop=(v == 3),
                    )

            def _emit_h3_evac(ps3_ref):
                # split halves: ScalarE + DVE concurrently, so ps3 (single
                # buffered) frees within one slot
                h3pack = h3pool.tile([D, J], bf16, tag="h3")
                nc.scalar.activation(
                    h3pack[:, 0:JC], ps3_ref[:, 0:JC], RELU, bias=b3_sb
                )
                nc.vector.tensor_scalar(
                    h3pack[:, JC:J], ps3_ref[:, JC:J], b3_sb, 0.0, ADD, MAX
                )
                return h3pack

            h1q = {gp: emit_h1(gp) for gp in range(LOOKAHEAD)}
            pend_l4 = None  # (g, h3pack) awaiting L4 emission
            pend_l3 = None  # (t, v, h2pack, ps3_ref, end_of_group g or None)

            for g in range(8):
                ps3_t = ps3.tile([D, J], mybir.dt.float32, tag="ps3")
                for p in range(8):
                    gp = g * 8 + p
                    t, v = p // 4, p % 4
                    if gp + LOOKAHEAD < NPAIR:
                        h1q[gp + LOOKAHEAD] = emit_h1(gp + LOOKAHEAD)
                    h1e, h1o = h1q.pop(gp)
                    ps2_t = ps2.tile([D, J], mybir.dt.float32, tag="ps2")
                    for jc in range(2):
                        jsl = slice(JC * jc, JC * jc + JC)
                        nc.tensor.matmul(
                            ps2_t[0:64, jsl], W2_sb, h1e[:, jsl],
                            tile_position=(0, 0),
                        )
                        nc.tensor.matmul(
                            ps2_t[64:128, jsl], W2_sb, h1o[:, jsl],
                            tile_position=(0, 64),
                        )
                    if pend_l4 is not None and p == 2:
                        _emit_l4(*pend_l4)
                        pend_l4 = None
                    if g == 4 and p == 3:
                        # rows 0:64 of the output are complete; evacuate on
                        # the DVE (the only engine with burst slack here)
                        for jc in range(2):
                            jsl = slice(JC * jc, JC * jc + JC)
                            nc.vector.tensor_scalar(
                                out_sb[0:64, jsl], out_ps[0:64, jsl],
                                b4_sb[0:64, :], None, ADD,
                            )
                            nc.sync.dma_start(
                                outd[0:64, jsl], out_sb[0:64, jsl]
                            )
                    h2pack = h2pool.tile([D, J], bf16, tag="h2")
                    if gp % EVAC_DVE_MOD == EVAC_DVE_PHASE:
                        nc.scalar.activation(
                            h2pack[:, 0:JC], ps2_t[:, 0:JC], RELU, bias=b2_sb
                        )
                        nc.vector.tensor_scalar(
                            h2pack[:, JC:J], ps2_t[:, JC:J], b2_sb, 0.0, ADD, MAX
                        )
                    else:
                        nc.scalar.activation(
                            h2pack[:], ps2_t[:], RELU, bias=b2_sb
                        )
                    # deferred L3 of the previous pair (runs after this
                    # pair's L2 on the PE, by which time its h2 evacuation
                    # has completed -> no PE stall)
                    if pend_l3 is not None:
                        pt, pv, ph2, pps3, pg_done = pend_l3
                        _emit_l3(pt, pv, ph2, pps3)
                        if pg_done is not None:
                            h3pack = _emit_h3_evac(pps3)
                            pend_l4 = (pg_done, h3pack)
                    pend_l3 = (t, v, h2pack, ps3_t, g if p == 7 else None)

            # epilogue: drain deferred work
            pt, pv, ph2, pps3, pg_done = pend_l3
            _emit_l3(pt, pv, ph2, pps3)
            h3pack = _emit_h3_evac(pps3)
            _emit_l4(pg_done, h3pack)

            # rows 64:128: split across ScalarE / DVE halves
            nc.scalar.activation(
                out_sb[64:128, 0:JC], out_ps[64:128, 0:JC], IDENT,
                bias=b4_sb[64:128, :],
            )
            nc.vector.tensor_scalar(
                out_sb[64:128, JC:J], out_ps[64:128, JC:J],
                b4_sb[64:128, :], None, ADD,
            )
            nc.sync.dma_start(outd[64:128, 0:JC], out_sb[64:128, 0:JC])
            nc.sync.dma_start(outd[64:128, JC:J], out_sb[64:128, JC:J])

    nc.compile()
    return nc


def _get_compiled():
    if "nc" not in _CACHE:
        _CACHE["nc"] = _build_bass()
    return _CACHE["nc"]


def _prep_in_maps(x, y, W1, b1, W2, b2, W3, b3, W4, b4):
    d = x.shape[1]
    W1a = W1[:d]
    W1b = W1[d:]
    W3P, W4P = _build_packed_weights(W3, W4)

    finpack = np.empty((D, 4), np.float32)
    finpack[:, F_B1] = b1
    finpack[:, F_B2] = np.concatenate([b2, b2])
    finpack[:, F_B3] = np.tile(b3, 16)
    finpack[:, F_B4] = b4[0]

    bin2p = np.empty((D, B_TOT), BF16)
    bin2p[:, B_W1B : B_W1B + D] = W1b.astype(BF16)
    bin2p[:, B_YT : B_YT + J] = y.T.astype(BF16)

    bin3p = np.empty((D, C_TOT), BF16)
    bin3p[:, C_W2 : C_W2 + 64] = W2.astype(BF16)
    bin3p[:, C_W3P : C_W3P + 256] = (
        W3P.transpose(1, 0, 2).reshape(D, 256).astype(BF16)
    )
    bin3p[:, C_W4P : C_W4P + 256] = (
        W4P.transpose(1, 0, 2).reshape(D, 256).astype(BF16)
    )

    W1a_bf = W1a.astype(BF16)
    in_maps = []
    for c in range(N_CORES):
        bin1p = np.empty((D, A_TOT), BF16)
        bin1p[:, A_W1A : A_W1A + D] = W1a_bf
        bin1p[:, A_XT : A_XT + NI] = x[c * NI : (c + 1) * NI].T.astype(BF16)
        in_maps.append(
            {"bin1": bin1p, "bin2": bin2p, "bin3": bin3p, "fin": finpack}
        )
    return in_maps


def run(x, y, W1, b1, W2, b2, W3, b3, W4, b4, **spmd_kwargs):
    """Run the kernel, returning (output, BassKernelResults)."""
    args = [np.asarray(a, np.float32) for a in
            (x, y, W1, b1, W2, b2, W3, b3, W4, b4)]
    in_maps = _prep_in_maps(*args)
    nc = _get_compiled()
    res = run_bass_kernel_spmd(nc, in_maps, list(range(N_CORES)), **spmd_kwargs)
    out = np.concatenate([np.asarray(r["out"]) for r in res.results], axis=0)
    return out.astype(np.float32), res


def kernel(x, y, W1, b1, W2, b2, W3, b3, W4, b4):
    out, _ = run(x, y, W1, b1, W2, b2, W3, b3, W4, b4)
    return out


# revision 4
# speedup vs baseline: 9.4099x; 9.4099x over previous
"""ConcatCritic MLP on 8 Trainium2 NeuronCores.

Computes out[a, b] = f(concat(x[a], y[b])) for a tiny 4-layer MLP
(256->128->64->8->1 with ReLU), i.e. a [1024, 1024] score matrix.

Sharding (per spec hint): x's batch dim across the 8 cores (128 rows each);
y and the weights replicated. Each core computes a [128, 1024] output block.

Dataflow per core (feature-on-partition layout):
  - Split layer 1: concat(x,y) @ W1 = x @ W1[:128] + y @ W1[128:].
    xab[f, i] = (W1a^T @ x^T)[f, i] + b1[f]   (128 x 128, fp32)
    ybT[f, j] = (W1b^T @ y^T)[f, j]           (128 x 1024, bf16)
  - h1_i = relu(ybT + xab[:, i]) -- one tensor_scalar per i (bf16); even i
    on the DVE, odd i on GpSimd (SBUF-only op, so GpSimd is legal and it
    offloads the DVE, which also handles PSUM-side evacuation spill).
  - L2: PE matmul, stationary W2 [128, 64]; even i -> PSUM rows 0:64
    (tile_position (0,0)), odd i -> rows 64:128 ((0,64)); one PSUM tile
    holds a pair of i's -> relu(+b2) evacuates [128, 1024] at once
    (ScalarE, with every 4th pair's second half on the DVE).
  - L3: stationary [128, 64] zero-padded 16-col strips (variant v for pair
    p = 4t+v) accumulate 4 pairs into each 64-row PSUM half; 8 pairs fill a
    dense [128, 1024] "h3pack" (16 i's) -> relu(+b3) evacuates split
    across ScalarE/DVE halves.
  - L4: stationary [128, 64] with one W4 entry per (row-block, i) strip;
    8 groups accumulate into one [128, 1024] PSUM = the core's full output
    block (+b4 on evacuation).

Pipelining: the PE is the roofline engine (~74us of matmul exec), so its
queue must never wait on an evacuation. Both L3 and L4 are DEFERRED: the
PE program order per pair-slot is [L2(p) x4][L4(g-1) at p==2][L3(p-1) x2],
so every matmul's input evacuation completed during the previous slot.
h1 production runs LOOKAHEAD pairs ahead.

All matmuls are bf16 (2 cols/cycle) in 128x64 column-tiling mode
(tile_positions (0,0)/(0,64) only) so the PE never mode-switches. PSUM
accumulation is fp32. x^T/y^T are cast to bf16 on the HOST and arrive in
dependency-ordered DMA chunks so layer 1 starts ~0.2us after launch.
PSUM budget: ps2 2x[128,1024] (4 banks) + ps3 1x (2) + pso 1x (2) = 8.
"""

import numpy as np
import ml_dtypes

import concourse.bass as bass
import concourse.bacc as bacc
import concourse.mybir as mybir
import concourse.tile as tile
from concourse.bass_utils import run_bass_kernel_spmd

BF16 = ml_dtypes.bfloat16
N_CORES = 8
B = 1024
D = 128
NI = B // N_CORES  # 128 rows of x per core
J = B              # full y batch per core
JC = 512           # matmul free-dim chunk (one PSUM bank)

# bin1 [128, 256] bf16: W1A | xT shard     (critical path: layer-1 x side)
A_W1A = 0
A_XT = A_W1A + D
A_TOT = A_XT + NI
# bin2 [128, 1152] bf16: W1B | yT          (layer-1 y side)
B_W1B = 0
B_YT = B_W1B + D
B_TOT = B_YT + J
# bin3 [128, 576] bf16: W2 | W3P | W4P     (needed from first L2 on)
C_W2 = 0
C_W3P = C_W2 + 64
C_W4P = C_W3P + 256
C_TOT = C_W4P + 256
# fin [128, 4] fp32: b1 | b2(x2) | b3(x16) | b4
F_B1, F_B2, F_B3, F_B4 = 0, 1, 2, 3

# h2 evacuation: pairs with (gp % EVAC_DVE_MOD) == EVAC_DVE_PHASE get their
# second column half evacuated by the DVE instead of ScalarE, keeping the
# ScalarE (whose steady load ~= the PE's) from pacing the kernel.
EVAC_DVE_MOD = 1000  # disabled: DVE is loaded with h1 + h3 halves
EVAC_DVE_PHASE = 2

LOOKAHEAD = 3
NPAIR = 64

_CACHE = {}


def _i_local_of_row(r):
    # h3pack row r -> which of the group's 16 i's it holds
    t, v, b = r // 64, (r % 64) // 16, (r % 16) // 8
    return 2 * (4 * t + v) + b


def _build_packed_weights(W3, W4):
    W3P = np.zeros((4, 128, 64), np.float32)
    for v in range(4):
        W3P[v, 0:64, 16 * v : 16 * v + 8] = W3
        W3P[v, 64:128, 16 * v + 8 : 16 * v + 16] = W3
    W4P = np.zeros((4, 128, 64), np.float32)
    for v4 in range(4):
        for r in range(128):
            c = 16 * v4 + _i_local_of_row(r)
            W4P[v4, r, c] = W4[r % 8, 0]
    return W3P, W4P


def _build_bass():
    nc = bacc.Bacc("TRN2", target_bir_lowering=False)
    f32 = mybir.dt.float32
    bf16 = mybir.dt.bfloat16

    bin1 = nc.dram_tensor("bin1", [D, A_TOT], bf16, kind="ExternalInput")
    bin2 = nc.dram_tensor("bin2", [D, B_TOT], bf16, kind="ExternalInput")
    bin3 = nc.dram_tensor("bin3", [D, C_TOT], bf16, kind="ExternalInput")
    fin_d = nc.dram_tensor("fin", [D, 4], f32, kind="ExternalInput")
    outd = nc.dram_tensor("out", [NI, J], f32, kind="ExternalOutput")

    RELU = mybir.ActivationFunctionType.Relu
    IDENT = mybir.ActivationFunctionType.Identity
    ADD = mybir.AluOpType.add
    MAX = mybir.AluOpType.max

    with tile.TileContext(nc) as tc:
        with (
            tc.tile_pool(name="const", bufs=1) as cpool,
            tc.tile_pool(name="work", bufs=10) as work,
            tc.tile_pool(name="h2p", bufs=6) as h2pool,
            tc.tile_pool(name="h3p", bufs=2) as h3pool,
            tc.tile_pool(name="ps2", bufs=2, space="PSUM") as ps2,
            tc.tile_pool(name="ps3", bufs=1, space="PSUM") as ps3,
            tc.tile_pool(name="pso", bufs=1, space="PSUM") as pso,
        ):
            hin1 = cpool.tile([D, A_TOT], bf16)
            hin2 = cpool.tile([D, B_TOT], bf16)
            hin3 = cpool.tile([D, C_TOT], bf16)
            fin = cpool.tile([D, 4], f32)
            xab = cpool.tile([D, NI], f32)
            ybT = cpool.tile([D, J], bf16)

            # dependency-ordered input DMAs (critical first)
            nc.sync.dma_start(hin1[:], bin1[:])
            nc.sync.dma_start(fin[:], fin_d[:])
            nc.sync.dma_start(hin2[:], bin2[:])
            nc.sync.dma_start(hin3[:], bin3[:])

            W2_sb = hin3[:, C_W2 : C_W2 + 64]
            b1_sb = fin[:, F_B1 : F_B1 + 1]
            b2_sb = fin[:, F_B2 : F_B2 + 1]
            b3_sb = fin[:, F_B3 : F_B3 + 1]
            b4_sb = fin[:, F_B4 : F_B4 + 1]

            # ---- precompute: xab = W1a^T @ xT + b1 ; ybT = W1b^T @ yT ----
            xa_ps = ps2.tile([D, J], mybir.dt.float32, tag="ps2")
            for c in range(2):
                nc.tensor.matmul(
                    xa_ps[64 * c : 64 * c + 64, :NI],
                    hin1[:, A_W1A + 64 * c : A_W1A + 64 * c + 64],
                    hin1[:, A_XT : A_XT + NI],
                    tile_position=(0, 64 * c),
                )
            nc.scalar.activation(xab[:], xa_ps[:, :NI], IDENT, bias=b1_sb)

            yb_ps = ps2.tile([D, J], mybir.dt.float32, tag="ps2")
            for c in range(2):
                for jc in range(2):
                    nc.tensor.matmul(
                        yb_ps[64 * c : 64 * c + 64, JC * jc : JC * jc + JC],
                        hin2[:, B_W1B + 64 * c : B_W1B + 64 * c + 64],
                        hin2[:, B_YT + JC * jc : B_YT + JC * jc + JC],
                        tile_position=(0, 64 * c),
                    )
            nc.scalar.activation(ybT[:], yb_ps[:], IDENT)

            # ---- main loop (software-pipelined emission) ----
            out_ps = pso.tile([D, J], mybir.dt.float32)
            out_sb = cpool.tile([NI, J], f32)

            def x_col(gp):
                g, p = gp // 8, gp % 8
                return 64 * (g // 4) + 16 * (g % 4) + 2 * p

            def emit_h1(gp):
                h1e = work.tile([D, J], bf16, tag="h1")
                h1o = work.tile([D, J], bf16, tag="h1")
                ie = x_col(gp)
                nc.vector.tensor_scalar(
                    h1e[:], ybT[:], xab[:, ie : ie + 1], 0.0, ADD, MAX
                )
                nc.vector.tensor_scalar(
                    h1o[:], ybT[:], xab[:, ie + 1 : ie + 2], 0.0, ADD, MAX
                )
                return h1e, h1o

            def _emit_l4(g, h3pack):
                t4, v4 = g // 4, g % 4
                for jc in range(2):
                    jsl = slice(JC * jc, JC * jc + JC)
                    nc.tensor.matmul(
                        out_ps[64 * t4 : 64 * t4 + 64, jsl],
                        hin3[:, C_W4P + 64 * v4 : C_W4P + 64 * v4 + 64],
                        h3pack[:, jsl],
                        tile_position=(0, 64 * t4),
                        start=(v4 == 0),
                        stop=(v4 == 3),
                    )

            def _emit_l3(t, v, h2pack, ps3_ref):
                for jc in range(2):
                    jsl = slice(JC * jc, JC * jc + JC)
                    nc.tensor.matmul(
                        ps3_ref[64 * t : 64 * t + 64, jsl],
                        hin3[:, C_W3P + 64 * v : C_W3P + 64 * v + 64],
                        h2pack[:, jsl],
                        tile_position=(0, 64 * t),
                        start=(v == 0),
                        stop=(v == 3),
                    )

            def _emit_h3_evac(ps3_ref):
                # split halves: ScalarE + DVE concurrently, so ps3 (single
                # buffered) frees within one slot
                h3pack = h3pool.tile([D, J], bf16, tag="h3")
                nc.scalar.activation(
                    h3pack[:, 0:JC], ps3_ref[:, 0:JC], RELU, bias=b3_sb
                )
                nc.vector.tensor_scalar(
                    h3pack[:, JC:J], ps3_ref[:, JC:J], b3_sb, 0.0, ADD, MAX
                )
                return h3pack

            h1q = {gp: emit_h1(gp) for gp in range(LOOKAHEAD)}
            pend_l4 = None  # (g, h3pack) awaiting L4 emission
            pend_l3 = None  # (t, v, h2pack, ps3_ref, end_of_group g or None)

            for g in range(8):
                ps3_t = ps3.tile([D, J], mybir.dt.float32, tag="ps3")
                for p in range(8):
                    gp = g * 8 + p
                    t, v = p // 4, p % 4
                    if gp + LOOKAHEAD < NPAIR:
                        h1q[gp + LOOKAHEAD] = emit_h1(gp + LOOKAHEAD)
                    h1e, h1o = h1q.pop(gp)
                    ps2_t = ps2.tile([D, J], mybir.dt.float32, tag="ps2")
                    for jc in range(2):
                        jsl = slice(JC * jc, JC * jc + JC)
                        nc.tensor.matmul(
                            ps2_t[0:64, jsl], W2_sb, h1e[:, jsl],
                            tile_position=(0, 0),
                        )
                        nc.tensor.matmul(
                            ps2_t[64:128, jsl], W2_sb, h1o[:, jsl],
                            tile_position=(0, 64),
                        )
                    if pend_l4 is not None and p == 2:
                        _emit_l4(*pend_l4)
                        pend_l4 = None
                    if g == 4 and p == 3:
                        # rows 0:64 of the output are complete; evacuate on
                        # the DVE (the only engine with burst slack here)
                        for jc in range(2):
                            jsl = slice(JC * jc, JC * jc + JC)
                            nc.vector.tensor_scalar(
                                out_sb[0:64, jsl], out_ps[0:64, jsl],
                                b4_sb[0:64, :], None, ADD,
                            )
                            nc.sync.dma_start(
                                outd[0:64, jsl], out_sb[0:64, jsl]
                            )
                    h2pack = h2pool.tile([D, J], bf16, tag="h2")
                    if gp % EVAC_DVE_MOD == EVAC_DVE_PHASE:
                        nc.scalar.activation(
                            h2pack[:, 0:JC], ps2_t[:, 0:JC], RELU, bias=b2_sb
                        )
                        nc.vector.tensor_scalar(
                            h2pack[:, JC:J], ps2_t[:, JC:J], b2_sb, 0.0, ADD, MAX
                        )
                    else:
                        nc.scalar.activation(
                            h2pack[:], ps2_t[:], RELU, bias=b2_sb
                        )
                    # deferred L3 of the previous pair (runs after this
                    # pair's L2 on the PE, by which time its h2 evacuation
                    # has completed -> no PE stall)
                    if pend_l3 is not None:
                        pt, pv, ph2, pps3, pg_done = pend_l3
                        _emit_l3(pt, pv, ph2, pps3)
                        if pg_done is not None:
                            h3pack = _emit_h3_evac(pps3)
                            pend_l4 = (pg_done, h3pack)
                    pend_l3 = (t, v, h2pack, ps3_t, g if p == 7 else None)

            # epilogue: drain deferred work
            pt, pv, ph2, pps3, pg_done = pend_l3
            _emit_l3(pt, pv, ph2, pps3)
            h3pack = _emit_h3_evac(pps3)
            _emit_l4(pg_done, h3pack)

            # rows 64:128: split across ScalarE / DVE halves
            nc.scalar.activation(
                out_sb[64:128, 0:JC], out_ps[64:128, 0:JC], IDENT,
                bias=b4_sb[64:128, :],
            )
            nc.vector.tensor_scalar(
                out_sb[64:128, JC:J], out_ps[64:128, JC:J],
                b4_sb[64:128, :], None, ADD,
            )
            nc.sync.dma_start(outd[64:128, 0:JC], out_sb[64:128, 0:JC])
            nc.sync.dma_start(outd[64:128, JC:J], out_sb[64:128, JC:J])

    nc.compile()
    return nc


def _get_compiled():
    if "nc" not in _CACHE:
        _CACHE["nc"] = _build_bass()
    return _CACHE["nc"]


def _prep_in_maps(x, y, W1, b1, W2, b2, W3, b3, W4, b4):
    d = x.shape[1]
    W1a = W1[:d]
    W1b = W1[d:]
    W3P, W4P = _build_packed_weights(W3, W4)

    finpack = np.empty((D, 4), np.float32)
    finpack[:, F_B1] = b1
    finpack[:, F_B2] = np.concatenate([b2, b2])
    finpack[:, F_B3] = np.tile(b3, 16)
    finpack[:, F_B4] = b4[0]

    bin2p = np.empty((D, B_TOT), BF16)
    bin2p[:, B_W1B : B_W1B + D] = W1b.astype(BF16)
    bin2p[:, B_YT : B_YT + J] = y.T.astype(BF16)

    bin3p = np.empty((D, C_TOT), BF16)
    bin3p[:, C_W2 : C_W2 + 64] = W2.astype(BF16)
    bin3p[:, C_W3P : C_W3P + 256] = (
        W3P.transpose(1, 0, 2).reshape(D, 256).astype(BF16)
    )
    bin3p[:, C_W4P : C_W4P + 256] = (
        W4P.transpose(1, 0, 2).reshape(D, 256).astype(BF16)
    )

    W1a_bf = W1a.astype(BF16)
    in_maps = []
    for c in range(N_CORES):
        bin1p = np.empty((D, A_TOT), BF16)
        bin1p[:, A_W1A : A_W1A + D] = W1a_bf
        bin1p[:, A_XT : A_XT + NI] = x[c * NI : (c + 1) * NI].T.astype(BF16)
        in_maps.append(
            {"bin1": bin1p, "bin2": bin2p, "bin3": bin3p, "fin": finpack}
        )
    return in_maps


def run(x, y, W1, b1, W2, b2, W3, b3, W4, b4, **spmd_kwargs):
    """Run the kernel, returning (output, BassKernelResults)."""
    args = [np.asarray(a, np.float32) for a in
            (x, y, W1, b1, W2, b2, W3, b3, W4, b4)]
    in_maps = _prep_in_maps(*args)
    nc = _get_compiled()
    res = run_bass_kernel_spmd(nc, in_maps, list(range(N_CORES)), **spmd_kwargs)
    out = np.concatenate([np.asarray(r["out"]) for r in res.results], axis=0)
    return out.astype(np.float32), res


def kernel(x, y, W1, b1, W2, b2, W3, b3, W4, b4):
    out, _ = run(x, y, W1, b1, W2, b2, W3, b3, W4, b4)
    return out


# revision 12
# speedup vs baseline: 9.4357x; 1.0027x over previous
"""ConcatCritic MLP on 8 Trainium2 NeuronCores.

Computes out[a, b] = f(concat(x[a], y[b])) for a tiny 4-layer MLP
(256->128->64->8->1 with ReLU), i.e. a [1024, 1024] score matrix.

Sharding (per spec hint): x's batch dim across the 8 cores (128 rows each);
y and the weights replicated. Each core computes a [128, 1024] output block.

Dataflow per core (feature-on-partition layout):
  - Split layer 1: concat(x,y) @ W1 = x @ W1[:128] + y @ W1[128:].
    xab[f, i] = (W1a^T @ x^T)[f, i] + b1[f]   (128 x 128, fp32)
    ybT[f, j] = (W1b^T @ y^T)[f, j]           (128 x 1024, bf16)
  - h1_i = relu(ybT + xab[:, i]) -- one tensor_scalar per i (bf16); even i
    on the DVE, odd i on GpSimd (SBUF-only op, so GpSimd is legal and it
    offloads the DVE, which also handles PSUM-side evacuation spill).
  - L2: PE matmul, stationary W2 [128, 64]; even i -> PSUM rows 0:64
    (tile_position (0,0)), odd i -> rows 64:128 ((0,64)); one PSUM tile
    holds a pair of i's -> relu(+b2) evacuates [128, 1024] at once
    (ScalarE, with every 4th pair's second half on the DVE).
  - L3: stationary [128, 64] zero-padded 16-col strips (variant v for pair
    p = 4t+v) accumulate 4 pairs into each 64-row PSUM half; 8 pairs fill a
    dense [128, 1024] "h3pack" (16 i's) -> relu(+b3) evacuates split
    across ScalarE/DVE halves.
  - L4: stationary [128, 64] with one W4 entry per (row-block, i) strip;
    8 groups accumulate into one [128, 1024] PSUM = the core's full output
    block (+b4 on evacuation).

Pipelining: the PE is the roofline engine (~74us of matmul exec), so its
queue must never wait on an evacuation. Both L3 and L4 are DEFERRED: the
PE program order per pair-slot is [L2(p) x4][L4(g-1) at p==2][L3(p-1) x2],
so every matmul's input evacuation completed during the previous slot.
h1 production runs LOOKAHEAD pairs ahead.

All matmuls are bf16 (2 cols/cycle) in 128x64 column-tiling mode
(tile_positions (0,0)/(0,64) only) so the PE never mode-switches. PSUM
accumulation is fp32. x^T/y^T are cast to bf16 on the HOST and arrive in
dependency-ordered DMA chunks so layer 1 starts ~0.2us after launch.
PSUM budget: ps2 2x[128,1024] (4 banks) + ps3 1x (2) + pso 1x (2) = 8.
"""

import numpy as np
import ml_dtypes

import concourse.bass as bass
import concourse.bacc as bacc
import concourse.mybir as mybir
import concourse.tile as tile
from concourse.bass_utils import run_bass_kernel_spmd

BF16 = ml_dtypes.bfloat16
N_CORES = 8
B = 1024
D = 128
NI = B // N_CORES  # 128 rows of x per core
J = B              # full y batch per core
JC = 512           # matmul free-dim chunk (one PSUM bank)

# bin1 [128, 256] bf16: W1A | xT shard     (critical path: layer-1 x side)
A_W1A = 0
A_XT = A_W1A + D
A_TOT = A_XT + NI
# bin2 [128, 1152] bf16: W1B | yT          (layer-1 y side)
B_W1B = 0
B_YT = B_W1B + D
B_TOT = B_YT + J
# bin3 [128, 576] bf16: W2 | W3P | W4P     (needed from first L2 on)
C_W2 = 0
C_W3P = C_W2 + 64
C_W4P = C_W3P + 256
C_TOT = C_W4P + 256
# fin [128, 4] fp32: b1 | b2(x2) | b3(x16) | b4
F_B1, F_B2, F_B3, F_B4 = 0, 1, 2, 3

# h2 evacuation: pairs with (gp % EVAC_DVE_MOD) == EVAC_DVE_PHASE get their
# second column half evacuated by the DVE instead of ScalarE, keeping the
# ScalarE (whose steady load ~= the PE's) from pacing the kernel.
EVAC_DVE_MOD = 1000  # disabled: DVE is loaded with h1 + h3 halves
EVAC_DVE_PHASE = 999

LOOKAHEAD = 3
NPAIR = 64

_CACHE = {}


def _i_local_of_row(r):
    # h3pack row r -> which of the group's 16 i's it holds
    t, v, b = r // 64, (r % 64) // 16, (r % 16) // 8
    return 2 * (4 * t + v) + b


def _build_packed_weights(W3, W4):
    W3P = np.zeros((4, 128, 64), np.float32)
    for v in range(4):
        W3P[v, 0:64, 16 * v : 16 * v + 8] = W3
        W3P[v, 64:128, 16 * v + 8 : 16 * v + 16] = W3
    W4P = np.zeros((4, 128, 64), np.float32)
    for v4 in range(4):
        for r in range(128):
            c = 16 * v4 + _i_local_of_row(r)
            W4P[v4, r, c] = W4[r % 8, 0]
    return W3P, W4P


def _build_bass():
    nc = bacc.Bacc("TRN2", target_bir_lowering=False)
    f32 = mybir.dt.float32
    bf16 = mybir.dt.bfloat16

    bin1 = nc.dram_tensor("bin1", [D, A_TOT], bf16, kind="ExternalInput")
    bin2 = nc.dram_tensor("bin2", [D, B_TOT], bf16, kind="ExternalInput")
    bin3 = nc.dram_tensor("bin3", [D, C_TOT], bf16, kind="ExternalInput")
    fin_d = nc.dram_tensor("fin", [D, 4], f32, kind="ExternalInput")
    outd = nc.dram_tensor("out", [NI, J], f32, kind="ExternalOutput")

    RELU = mybir.ActivationFunctionType.Relu
    IDENT = mybir.ActivationFunctionType.Identity
    ADD = mybir.AluOpType.add
    MAX = mybir.AluOpType.max

    with tile.TileContext(nc) as tc:
        with (
            tc.tile_pool(name="const", bufs=1) as cpool,
            tc.tile_pool(name="work", bufs=14) as work,
            tc.tile_pool(name="h2p", bufs=6) as h2pool,
            tc.tile_pool(name="h3p", bufs=2) as h3pool,
            tc.tile_pool(name="ps2", bufs=2, space="PSUM") as ps2,
            tc.tile_pool(name="ps3", bufs=1, space="PSUM") as ps3,
            tc.tile_pool(name="pso", bufs=1, space="PSUM") as pso,
        ):
            hin1 = cpool.tile([D, A_TOT], bf16)
            hin2 = cpool.tile([D, B_TOT], bf16)
            hin3 = cpool.tile([D, C_TOT], bf16)
            fin = cpool.tile([D, 4], f32)
            xab = cpool.tile([D, NI], f32)
            ybT = cpool.tile([D, J], bf16)
            scratch = cpool.tile([D, 1], f32)

            # dependency-ordered input DMAs (critical first)
            nc.sync.dma_start(hin1[:], bin1[:])
            nc.sync.dma_start(fin[:], fin_d[:])
            nc.sync.dma_start(hin2[:], bin2[:])
            nc.sync.dma_start(hin3[:], bin3[:])

            # dummy activation: pulls the ~1.3us ACT_TABLE_LOAD off the
            # critical path (runs while the input DMAs are in flight)
            nc.vector.memset(scratch[:], 0.0)
            nc.scalar.activation(scratch[:], scratch[:], RELU)

            W2_sb = hin3[:, C_W2 : C_W2 + 64]
            b1_sb = fin[:, F_B1 : F_B1 + 1]
            b2_sb = fin[:, F_B2 : F_B2 + 1]
            b3_sb = fin[:, F_B3 : F_B3 + 1]
            b4_sb = fin[:, F_B4 : F_B4 + 1]

            # ---- precompute: xab = W1a^T @ xT + b1 ; ybT = W1b^T @ yT ----
            # xa_ps borrows the pso pool (out_ps is its next generation;
            # its first write happens long after xab is evacuated)
            xa_ps = pso.tile([D, J], mybir.dt.float32, tag="pso")
            for c in range(2):
                nc.tensor.matmul(
                    xa_ps[64 * c : 64 * c + 64, :NI],
                    hin1[:, A_W1A + 64 * c : A_W1A + 64 * c + 64],
                    hin1[:, A_XT : A_XT + NI],
                    tile_position=(0, 64 * c),
                )
            nc.scalar.activation(xab[:], xa_ps[:, :NI], IDENT, bias=b1_sb)

            yb_ps = ps2.tile([D, J], mybir.dt.float32, tag="ps2")
            for jc in range(2):
                for c in range(2):
                    nc.tensor.matmul(
                        yb_ps[64 * c : 64 * c + 64, JC * jc : JC * jc + JC],
                        hin2[:, B_W1B + 64 * c : B_W1B + 64 * c + 64],
                        hin2[:, B_YT + JC * jc : B_YT + JC * jc + JC],
                        tile_position=(0, 64 * c),
                    )
            # evacuate ybT in column halves on both engines so the first
            # h1 tensor_scalars (which are also emitted per-half) start as
            # soon as half the columns are ready
            nc.scalar.activation(ybT[:, 0:JC], yb_ps[:, 0:JC], IDENT)
            nc.vector.tensor_scalar(
                ybT[:, JC:J], yb_ps[:, JC:J], 0.0, None, ADD
            )

            # ---- main loop (software-pipelined emission) ----
            out_ps = pso.tile([D, J], mybir.dt.float32, tag="pso")
            out_sb = cpool.tile([NI, J], f32)

            def x_col(gp):
                g, p = gp // 8, gp % 8
                return 64 * (g // 4) + 16 * (g % 4) + 2 * p

            def emit_h1(gp, half=None):
                h1e = work.tile([D, J], bf16, tag="h1")
                h1o = work.tile([D, J], bf16, tag="h1")
                ie = x_col(gp)
                sl = slice(0, J) if half is None else slice(JC * half, JC * half + JC)
                nc.vector.tensor_scalar(
                    h1e[:, sl], ybT[:, sl], xab[:, ie : ie + 1], 0.0, ADD, MAX
                )
                nc.vector.tensor_scalar(
                    h1o[:, sl], ybT[:, sl], xab[:, ie + 1 : ie + 2], 0.0, ADD, MAX
                )
                return h1e, h1o

            def emit_h1_half(gp, h1eo, half):
                h1e, h1o = h1eo
                ie = x_col(gp)
                sl = slice(JC * half, JC * half + JC)
                nc.vector.tensor_scalar(
                    h1e[:, sl], ybT[:, sl], xab[:, ie : ie + 1], 0.0, ADD, MAX
                )
                nc.vector.tensor_scalar(
                    h1o[:, sl], ybT[:, sl], xab[:, ie + 1 : ie + 2], 0.0, ADD, MAX
                )

            def _emit_l4(g, h3pack):
                t4, v4 = g // 4, g % 4
                for jc in range(2):
                    jsl = slice(JC * jc, JC * jc + JC)
                    nc.tensor.matmul(
                        out_ps[64 * t4 : 64 * t4 + 64, jsl],
                        hin3[:, C_W4P + 64 * v4 : C_W4P + 64 * v4 + 64],
                        h3pack[:, jsl],
                        tile_position=(0, 64 * t4),
                        start=(v4 == 0),
                        stop=(v4 == 3),
                    )

            def _emit_l3(t, v, h2pack, ps3_ref):
                for jc in range(2):
                    jsl = slice(JC * jc, JC * jc + JC)
                    nc.tensor.matmul(
                        ps3_ref[64 * t : 64 * t + 64, jsl],
                        hin3[:, C_W3P + 64 * v : C_W3P + 64 * v + 64],
                        h2pack[:, jsl],
                        tile_position=(0, 64 * t),
                        start=(v == 0),
                        stop=(v == 3),
                    )

            def _emit_h3_evac(ps3_ref):
                # split halves: ScalarE + DVE concurrently, so ps3 (single
                # buffered) frees within one slot
                h3pack = h3pool.tile([D, J], bf16, tag="h3")
                nc.scalar.activation(
                    h3pack[:, 0:JC], ps3_ref[:, 0:JC], RELU, bias=b3_sb
                )
                nc.vector.tensor_scalar(
                    h3pack[:, JC:J], ps3_ref[:, JC:J], b3_sb, 0.0, ADD, MAX
                )
                return h3pack

            # prefill: first two pairs emitted half-by-half (jc0 halves
            # first) so the first L2A matmuls start as soon as the jc0
            # column half of ybT is evacuated
            h1q = {0: emit_h1(0, half=0), 1: emit_h1(1, half=0)}
            emit_h1_half(0, h1q[0], 1)
            emit_h1_half(1, h1q[1], 1)
            h1q[2] = emit_h1(2)
            pend_l4 = None  # (g, h3pack) awaiting L4 emission
            pend_l3 = None  # (t, v, h2pack, ps3_ref, end_of_group g or None)

            for g in range(8):
                ps3_t = ps3.tile([D, J], mybir.dt.float32, tag="ps3")
                for p in range(8):
                    gp = g * 8 + p
                    t, v = p // 4, p % 4
                    h1e, h1o = h1q.pop(gp)
                    ps2_t = ps2.tile([D, J], mybir.dt.float32, tag="ps2")
                    for jc in range(2):
                        jsl = slice(JC * jc, JC * jc + JC)
                        nc.tensor.matmul(
                            ps2_t[0:64, jsl], W2_sb, h1e[:, jsl],
                            tile_position=(0, 0),
                        )
                        nc.tensor.matmul(
                            ps2_t[64:128, jsl], W2_sb, h1o[:, jsl],
                            tile_position=(0, 64),
                        )
                    # deferred L3 of the previous pair (runs after this
                    # pair's L2 on the PE, by which time its h2 evacuation
                    # has completed -> no PE stall). At a group boundary
                    # the h3 evacuation is emitted here, BEFORE this slot's
                    # h1/h2 work, so both its halves lead their engines'
                    # queues and ps3 (single-buffered) frees within a slot.
                    if pend_l3 is not None:
                        pt, pv, ph2, pps3, pg_done = pend_l3
                        _emit_l3(pt, pv, ph2, pps3)
                        if pg_done is not None:
                            h3pack = _emit_h3_evac(pps3)
                            pend_l4 = (pg_done, h3pack)
                    h2pack_cur = h2pool.tile([D, J], bf16, tag="h2")
                    pend_l3 = (t, v, h2pack_cur, ps3_t, g if p == 7 else None)
                    if pend_l4 is not None and p == 5:
                        _emit_l4(*pend_l4)
                        pend_l4 = None
                    if g == 4 and p in (6, 7):
                        # rows 0:64 of the output are complete (L4 of g3
                        # ran at p==5); evacuate one column half per slot
                        # on the DVE, which has the burst slack here
                        jsl = slice(JC * (p - 6), JC * (p - 6) + JC)
                        nc.vector.tensor_scalar(
                            out_sb[0:64, jsl], out_ps[0:64, jsl],
                            b4_sb[0:64, :], None, ADD,
                        )
                        nc.sync.dma_start(outd[0:64, jsl], out_sb[0:64, jsl])
                    if gp + LOOKAHEAD < NPAIR:
                        h1q[gp + LOOKAHEAD] = emit_h1(gp + LOOKAHEAD)
                    if gp % EVAC_DVE_MOD == EVAC_DVE_PHASE:
                        nc.scalar.activation(
                            h2pack_cur[:, 0:JC], ps2_t[:, 0:JC], RELU, bias=b2_sb
                        )
                        nc.vector.tensor_scalar(
                            h2pack_cur[:, JC:J], ps2_t[:, JC:J], b2_sb, 0.0, ADD, MAX
                        )
                    else:
                        nc.scalar.activation(
                            h2pack_cur[:], ps2_t[:], RELU, bias=b2_sb
                        )

            # epilogue: drain deferred work, pipelined at half-column
            # granularity across ScalarE (jc0) / DVE (jc1)
            pt, pv, ph2, pps3, pg_done = pend_l3
            _emit_l3(pt, pv, ph2, pps3)
            h3pack = h3pool.tile([D, J], bf16, tag="h3")
            nc.scalar.activation(
                h3pack[:, 0:JC], pps3[:, 0:JC], RELU, bias=b3_sb
            )
            nc.vector.tensor_scalar(
                h3pack[:, JC:J], pps3[:, JC:J], b3_sb, 0.0, ADD, MAX
            )
            t4, v4 = pg_done // 4, pg_done % 4
            for jc in range(2):
                jsl = slice(JC * jc, JC * jc + JC)
                nc.tensor.matmul(
                    out_ps[64 * t4 : 64 * t4 + 64, jsl],
                    hin3[:, C_W4P + 64 * v4 : C_W4P + 64 * v4 + 64],
                    h3pack[:, jsl],
                    tile_position=(0, 64 * t4),
                    start=(v4 == 0),
                    stop=(v4 == 3),
                )
                if jc == 0:
                    nc.scalar.activation(
                        out_sb[64:128, jsl], out_ps[64:128, jsl], IDENT,
                        bias=b4_sb[64:128, :],
                    )
                else:
                    nc.vector.tensor_scalar(
                        out_sb[64:128, jsl], out_ps[64:128, jsl],
                        b4_sb[64:128, :], None, ADD,
                    )
                nc.sync.dma_start(outd[64:128, jsl], out_sb[64:128, jsl])

    nc.compile()
    return nc


def _get_compiled():
    if "nc" not in _CACHE:
        _CACHE["nc"] = _build_bass()
    return _CACHE["nc"]


def _prep_in_maps(x, y, W1, b1, W2, b2, W3, b3, W4, b4):
    d = x.shape[1]
    W1a = W1[:d]
    W1b = W1[d:]
    W3P, W4P = _build_packed_weights(W3, W4)

    finpack = np.empty((D, 4), np.float32)
    finpack[:, F_B1] = b1
    finpack[:, F_B2] = np.concatenate([b2, b2])
    finpack[:, F_B3] = np.tile(b3, 16)
    finpack[:, F_B4] = b4[0]

    bin2p = np.empty((D, B_TOT), BF16)
    bin2p[:, B_W1B : B_W1B + D] = W1b.astype(BF16)
    bin2p[:, B_YT : B_YT + J] = y.T.astype(BF16)

    bin3p = np.empty((D, C_TOT), BF16)
    bin3p[:, C_W2 : C_W2 + 64] = W2.astype(BF16)
    bin3p[:, C_W3P : C_W3P + 256] = (
        W3P.transpose(1, 0, 2).reshape(D, 256).astype(BF16)
    )
    bin3p[:, C_W4P : C_W4P + 256] = (
        W4P.transpose(1, 0, 2).reshape(D, 256).astype(BF16)
    )

    W1a_bf = W1a.astype(BF16)
    in_maps = []
    for c in range(N_CORES):
        bin1p = np.empty((D, A_TOT), BF16)
        bin1p[:, A_W1A : A_W1A + D] = W1a_bf
        bin1p[:, A_XT : A_XT + NI] = x[c * NI : (c + 1) * NI].T.astype(BF16)
        in_maps.append(
            {"bin1": bin1p, "bin2": bin2p, "bin3": bin3p, "fin": finpack}
        )
    return in_maps


def run(x, y, W1, b1, W2, b2, W3, b3, W4, b4, **spmd_kwargs):
    """Run the kernel, returning (output, BassKernelResults)."""
    args = [np.asarray(a, np.float32) for a in
            (x, y, W1, b1, W2, b2, W3, b3, W4, b4)]
    in_maps = _prep_in_maps(*args)
    nc = _get_compiled()
    res = run_bass_kernel_spmd(nc, in_maps, list(range(N_CORES)), **spmd_kwargs)
    out = np.concatenate([np.asarray(r["out"]) for r in res.results], axis=0)
    return out.astype(np.float32), res


def kernel(x, y, W1, b1, W2, b2, W3, b3, W4, b4):
    out, _ = run(x, y, W1, b1, W2, b2, W3, b3, W4, b4)
    return out


# revision 21
# speedup vs baseline: 9.7982x; 1.0384x over previous
"""ConcatCritic MLP on 8 Trainium2 NeuronCores.

Computes out[a, b] = f(concat(x[a], y[b])) for a tiny 4-layer MLP
(256->128->64->8->1 with ReLU), i.e. a [1024, 1024] score matrix.

Sharding (per spec hint): x's batch dim across the 8 cores (128 rows each);
y and the weights replicated. Each core computes a [128, 1024] output block.

Dataflow per core (feature-on-partition layout):
  - Split layer 1: concat(x,y) @ W1 = x @ W1[:128] + y @ W1[128:].
    xab[f, i] = (W1a^T @ x^T)[f, i] + b1[f]   (128 x 128, fp32)
    ybT[f, j] = (W1b^T @ y^T)[f, j]           (128 x 1024, bf16)
  - h1_i = relu(ybT + xab[:, i]) -- one tensor_scalar per i (bf16); even i
    on the DVE, odd i on GpSimd (SBUF-only op, so GpSimd is legal and it
    offloads the DVE, which also handles PSUM-side evacuation spill).
  - L2: PE matmul, stationary W2 [128, 64]; even i -> PSUM rows 0:64
    (tile_position (0,0)), odd i -> rows 64:128 ((0,64)); one PSUM tile
    holds a pair of i's -> relu(+b2) evacuates [128, 1024] at once
    (ScalarE, with every 4th pair's second half on the DVE).
  - L3: stationary [128, 64] zero-padded 16-col strips (variant v for pair
    p = 4t+v) accumulate 4 pairs into each 64-row PSUM half; 8 pairs fill a
    dense [128, 1024] "h3pack" (16 i's) -> relu(+b3) evacuates split
    across ScalarE/DVE halves.
  - L4: stationary [128, 64] with one W4 entry per (row-block, i) strip;
    8 groups accumulate into one [128, 1024] PSUM = the core's full output
    block (+b4 on evacuation).

Pipelining: the PE is the roofline engine (~74us of matmul exec), so its
queue must never wait on an evacuation. Both L3 and L4 are DEFERRED: the
PE program order per pair-slot is [L2(p) x4][L4(g-1) at p==2][L3(p-1) x2],
so every matmul's input evacuation completed during the previous slot.
h1 production runs LOOKAHEAD pairs ahead.

All matmuls are bf16 (2 cols/cycle) in 128x64 column-tiling mode
(tile_positions (0,0)/(0,64) only) so the PE never mode-switches. PSUM
accumulation is fp32. x^T/y^T are cast to bf16 on the HOST and arrive in
dependency-ordered DMA chunks so layer 1 starts ~0.2us after launch.
PSUM budget: ps2 2x[128,1024] (4 banks) + ps3 1x (2) + pso 1x (2) = 8.
"""

import numpy as np
import ml_dtypes

import concourse.bass as bass
import concourse.bacc as bacc
import concourse.mybir as mybir
import concourse.tile as tile
from concourse.bass_utils import run_bass_kernel_spmd

BF16 = ml_dtypes.bfloat16
N_CORES = 8
B = 1024
D = 128
NI = B // N_CORES  # 128 rows of x per core
J = B              # full y batch per core
JC = 512           # matmul free-dim chunk (one PSUM bank)

# bin1 [128, 256] bf16: W1A | xT shard     (critical path: layer-1 x side)
A_W1A = 0
A_XT = A_W1A + D
A_TOT = A_XT + NI
# bin2 [128, 1152] bf16: W1B | yT          (layer-1 y side)
B_W1B = 0
B_YT = B_W1B + D
B_TOT = B_YT + J
# bin3 [128, 576] bf16: W2 | W3P | W4P     (needed from first L2 on)
C_W2 = 0
C_W3P = C_W2 + 64
C_W4P = C_W3P + 256
C_TOT = C_W4P + 256
# fin [128, 4] fp32: b1 | b2(x2) | b3(x16) | b4
F_B1, F_B2, F_B3, F_B4 = 0, 1, 2, 3

# h2 evacuation: pairs with (gp % EVAC_DVE_MOD) == EVAC_DVE_PHASE get their
# second column half evacuated by the DVE instead of ScalarE, keeping the
# ScalarE (whose steady load ~= the PE's) from pacing the kernel.
EVAC_DVE_MOD = 1000  # disabled: DVE is loaded with h1 + h3 halves
EVAC_DVE_PHASE = 999

LOOKAHEAD = 3
NPAIR = 64

_CACHE = {}


def _i_local_of_row(r):
    # h3pack row r -> which of the group's 16 i's it holds
    t, v, b = r // 64, (r % 64) // 16, (r % 16) // 8
    return 2 * (4 * t + v) + b


def _build_packed_weights(W3, W4):
    W3P = np.zeros((4, 128, 64), np.float32)
    for v in range(4):
        W3P[v, 0:64, 16 * v : 16 * v + 8] = W3
        W3P[v, 64:128, 16 * v + 8 : 16 * v + 16] = W3
    W4P = np.zeros((4, 128, 64), np.float32)
    for v4 in range(4):
        for r in range(128):
            c = 16 * v4 + _i_local_of_row(r)
            W4P[v4, r, c] = W4[r % 8, 0]
    return W3P, W4P


def _build_bass():
    nc = bacc.Bacc("TRN2", target_bir_lowering=False)
    f32 = mybir.dt.float32
    bf16 = mybir.dt.bfloat16

    bin1 = nc.dram_tensor("bin1", [D, A_TOT], bf16, kind="ExternalInput")
    bin2 = nc.dram_tensor("bin2", [D, B_TOT], bf16, kind="ExternalInput")
    bin3 = nc.dram_tensor("bin3", [D, C_TOT], bf16, kind="ExternalInput")
    fin_d = nc.dram_tensor("fin", [D, 4], f32, kind="ExternalInput")
    outd = nc.dram_tensor("out", [NI, J], f32, kind="ExternalOutput")

    RELU = mybir.ActivationFunctionType.Relu
    IDENT = mybir.ActivationFunctionType.Identity
    ADD = mybir.AluOpType.add
    MAX = mybir.AluOpType.max

    with tile.TileContext(nc) as tc:
        with (
            tc.tile_pool(name="const", bufs=1) as cpool,
            tc.tile_pool(name="work", bufs=14) as work,
            tc.tile_pool(name="h2p", bufs=6) as h2pool,
            tc.tile_pool(name="h3p", bufs=2) as h3pool,
            tc.tile_pool(name="ps2", bufs=2, space="PSUM") as ps2,
            tc.tile_pool(name="ps3", bufs=1, space="PSUM") as ps3,
            tc.tile_pool(name="pso", bufs=1, space="PSUM") as pso,
        ):
            hin1 = cpool.tile([D, A_TOT], bf16)
            hin2 = cpool.tile([D, B_TOT], bf16)
            hin3 = cpool.tile([D, C_TOT], bf16)
            fin = cpool.tile([D, 4], f32)
            xab = cpool.tile([D, NI], f32)
            ybT = cpool.tile([D, J], bf16)
            scratch = cpool.tile([D, 1], f32)

            # dependency-ordered input DMAs on BOTH hardware DMA queues
            # (Sync + ScalarE are the TRN2 HWDGE engines) so the x-side and
            # y-side layer-1 inputs land concurrently
            nc.sync.dma_start(hin1[:], bin1[:])
            nc.scalar.dma_start(hin2[:], bin2[:])
            nc.sync.dma_start(fin[:], fin_d[:])
            nc.scalar.dma_start(hin3[:], bin3[:])

            # dummy activation: pulls the ~1.3us ACT_TABLE_LOAD off the
            # critical path (runs while the input DMAs are in flight)
            nc.vector.memset(scratch[:], 0.0)
            nc.scalar.activation(scratch[:], scratch[:], RELU)

            W2_sb = hin3[:, C_W2 : C_W2 + 64]
            b1_sb = fin[:, F_B1 : F_B1 + 1]
            b2_sb = fin[:, F_B2 : F_B2 + 1]
            b3_sb = fin[:, F_B3 : F_B3 + 1]
            b4_sb = fin[:, F_B4 : F_B4 + 1]

            # ---- precompute: xab = W1a^T @ xT + b1 ; ybT = W1b^T @ yT ----
            # xa_ps borrows the pso pool (out_ps is its next generation;
            # its first write happens long after xab is evacuated)
            xa_ps = pso.tile([D, J], mybir.dt.float32, tag="pso")
            for c in range(2):
                nc.tensor.matmul(
                    xa_ps[64 * c : 64 * c + 64, :NI],
                    hin1[:, A_W1A + 64 * c : A_W1A + 64 * c + 64],
                    hin1[:, A_XT : A_XT + NI],
                    tile_position=(0, 64 * c),
                )
            nc.scalar.activation(xab[:], xa_ps[:, :NI], IDENT, bias=b1_sb)

            yb_ps = ps2.tile([D, J], mybir.dt.float32, tag="ps2")
            for jc in range(2):
                for c in range(2):
                    nc.tensor.matmul(
                        yb_ps[64 * c : 64 * c + 64, JC * jc : JC * jc + JC],
                        hin2[:, B_W1B + 64 * c : B_W1B + 64 * c + 64],
                        hin2[:, B_YT + JC * jc : B_YT + JC * jc + JC],
                        tile_position=(0, 64 * c),
                    )
            # evacuate ybT in column halves on both engines so the first
            # h1 tensor_scalars (which are also emitted per-half) start as
            # soon as half the columns are ready
            nc.scalar.activation(ybT[:, 0:JC], yb_ps[:, 0:JC], IDENT)
            nc.vector.tensor_scalar(
                ybT[:, JC:J], yb_ps[:, JC:J], 0.0, None, ADD
            )

            # ---- main loop (software-pipelined emission) ----
            out_ps = pso.tile([D, J], mybir.dt.float32, tag="pso")
            out_sb = cpool.tile([NI, J], f32)

            def x_col(gp):
                g, p = gp // 8, gp % 8
                return 64 * (g // 4) + 16 * (g % 4) + 2 * p

            def emit_h1(gp, half=None):
                h1e = work.tile([D, J], bf16, tag="h1")
                h1o = work.tile([D, J], bf16, tag="h1")
                ie = x_col(gp)
                sl = slice(0, J) if half is None else slice(JC * half, JC * half + JC)
                nc.vector.tensor_scalar(
                    h1e[:, sl], ybT[:, sl], xab[:, ie : ie + 1], 0.0, ADD, MAX
                )
                nc.vector.tensor_scalar(
                    h1o[:, sl], ybT[:, sl], xab[:, ie + 1 : ie + 2], 0.0, ADD, MAX
                )
                return h1e, h1o

            def emit_h1_half(gp, h1eo, half):
                h1e, h1o = h1eo
                ie = x_col(gp)
                sl = slice(JC * half, JC * half + JC)
                nc.vector.tensor_scalar(
                    h1e[:, sl], ybT[:, sl], xab[:, ie : ie + 1], 0.0, ADD, MAX
                )
                nc.vector.tensor_scalar(
                    h1o[:, sl], ybT[:, sl], xab[:, ie + 1 : ie + 2], 0.0, ADD, MAX
                )

            def _emit_l4(g, h3pack):
                t4, v4 = g // 4, g % 4
                for jc in range(2):
                    jsl = slice(JC * jc, JC * jc + JC)
                    nc.tensor.matmul(
                        out_ps[64 * t4 : 64 * t4 + 64, jsl],
                        hin3[:, C_W4P + 64 * v4 : C_W4P + 64 * v4 + 64],
                        h3pack[:, jsl],
                        tile_position=(0, 64 * t4),
                        start=(v4 == 0),
                        stop=(v4 == 3),
                    )

            def _emit_l3(t, v, h2pack, ps3_ref):
                for jc in range(2):
                    jsl = slice(JC * jc, JC * jc + JC)
                    nc.tensor.matmul(
                        ps3_ref[64 * t : 64 * t + 64, jsl],
                        hin3[:, C_W3P + 64 * v : C_W3P + 64 * v + 64],
                        h2pack[:, jsl],
                        tile_position=(0, 64 * t),
                        start=(v == 0),
                        stop=(v == 3),
                    )

            def _emit_h3_evac(ps3_ref):
                # split halves: ScalarE + DVE concurrently, so ps3 (single
                # buffered) frees within one slot
                h3pack = h3pool.tile([D, J], bf16, tag="h3")
                nc.scalar.activation(
                    h3pack[:, 0:JC], ps3_ref[:, 0:JC], RELU, bias=b3_sb
                )
                nc.vector.tensor_scalar(
                    h3pack[:, JC:J], ps3_ref[:, JC:J], b3_sb, 0.0, ADD, MAX
                )
                return h3pack

            # prefill: first two pairs emitted half-by-half (jc0 halves
            # first) so the first L2A matmuls start as soon as the jc0
            # column half of ybT is evacuated
            h1q = {0: emit_h1(0, half=0), 1: emit_h1(1, half=0)}
            emit_h1_half(0, h1q[0], 1)
            emit_h1_half(1, h1q[1], 1)
            h1q[2] = emit_h1(2)
            pend_l4 = None   # (g, h3pack) awaiting L4 emission
            pend_l3 = []     # [(target_gp, t, v, h2pack, ps3_ref, g_done)]
            pend_h3a = None  # ScalarE h3 half, emitted after this slot's h2

            def slot(gp, ps3_t):
                g, p = gp // 8, gp % 8
                nonlocal pend_l4, pend_h3a
                t, v = p // 4, p % 4
                h1e, h1o = h1q.pop(gp)
                ps2_t = ps2.tile([D, J], mybir.dt.float32, tag="ps2")
                for jc in range(2):
                    jsl = slice(JC * jc, JC * jc + JC)
                    nc.tensor.matmul(
                        ps2_t[0:64, jsl], W2_sb, h1e[:, jsl],
                        tile_position=(0, 0),
                    )
                    nc.tensor.matmul(
                        ps2_t[64:128, jsl], W2_sb, h1o[:, jsl],
                        tile_position=(0, 64),
                    )
                # deferred L3s whose target slot arrived (defer 2 for pairs
                # p0..p5, defer 1 for p6/p7 so a group's ps3 completes two
                # slots before the next group's first L3 reuses the banks)
                while pend_l3 and pend_l3[0][0] <= gp:
                    _, pt, pv, ph2, pps3, pg_done = pend_l3.pop(0)
                    _emit_l3(pt, pv, ph2, pps3)
                    if pg_done is not None:
                        # h3 evac: DVE half now (leads the DVE queue);
                        # ScalarE half after this slot's h2 evacuation
                        h3pack = h3pool.tile([D, J], bf16, tag="h3")
                        nc.vector.tensor_scalar(
                            h3pack[:, JC:J], pps3[:, JC:J], b3_sb, 0.0,
                            ADD, MAX,
                        )
                        pend_h3a = (pps3, h3pack)
                        pend_l4 = (pg_done, h3pack)
                h2pack_cur = h2pool.tile([D, J], bf16, tag="h2")
                pend_l3.append(
                    (gp + (1 if p >= 6 else 2), t, v, h2pack_cur, ps3_t,
                     g if p == 7 else None)
                )
                if pend_l4 is not None and p == 5:
                    _emit_l4(*pend_l4)
                    pend_l4 = None
                if g == 4 and p in (6, 7):
                    # rows 0:64 are final (L4 of g3 ran at p==5): evacuate
                    # one column half per slot on the DVE (+b4), then DMA
                    jsl = slice(JC * (p - 6), JC * (p - 6) + JC)
                    nc.vector.tensor_scalar(
                        out_sb[0:64, jsl], out_ps[0:64, jsl],
                        b4_sb[0:64, :], None, ADD,
                    )
                    nc.sync.dma_start(outd[0:64, jsl], out_sb[0:64, jsl])
                # one h2 spill per group (pair p1): its jc1 half goes to
                # the DVE, giving ScalarE slack to absorb the h3 burst
                spill = p == 1
                if spill:
                    nc.vector.tensor_scalar(
                        h2pack_cur[:, JC:J], ps2_t[:, JC:J], b2_sb, 0.0,
                        ADD, MAX,
                    )
                if gp + LOOKAHEAD < NPAIR:
                    h1q[gp + LOOKAHEAD] = emit_h1(gp + LOOKAHEAD)
                if spill:
                    nc.scalar.activation(
                        h2pack_cur[:, 0:JC], ps2_t[:, 0:JC], RELU, bias=b2_sb
                    )
                else:
                    nc.scalar.activation(
                        h2pack_cur[:], ps2_t[:], RELU, bias=b2_sb
                    )
                if pend_h3a is not None:
                    pps3, h3pack = pend_h3a
                    nc.scalar.activation(
                        h3pack[:, 0:JC], pps3[:, 0:JC], RELU, bias=b3_sb
                    )
                    pend_h3a = None

            for g in range(8):
                ps3_t = ps3.tile([D, J], mybir.dt.float32, tag="ps3")
                for p in range(8):
                    slot(g * 8 + p, ps3_t)

            # epilogue: drain deferred L3s (p6, p7), h3 evac, final L4,
            # then DMA rows 64:128 straight from PSUM per column half
            while pend_l3:
                _, pt, pv, ph2, pps3, pg_done = pend_l3.pop(0)
                _emit_l3(pt, pv, ph2, pps3)
            h3pack = h3pool.tile([D, J], bf16, tag="h3")
            nc.scalar.activation(
                h3pack[:, 0:JC], pps3[:, 0:JC], RELU, bias=b3_sb
            )
            nc.vector.tensor_scalar(
                h3pack[:, JC:J], pps3[:, JC:J], b3_sb, 0.0, ADD, MAX
            )
            t4, v4 = pg_done // 4, pg_done % 4
            for jc in range(2):
                jsl = slice(JC * jc, JC * jc + JC)
                nc.tensor.matmul(
                    out_ps[64 * t4 : 64 * t4 + 64, jsl],
                    hin3[:, C_W4P + 64 * v4 : C_W4P + 64 * v4 + 64],
                    h3pack[:, jsl],
                    tile_position=(0, 64 * t4),
                    start=(v4 == 0),
                    stop=(v4 == 3),
                )
                if jc == 0:
                    nc.scalar.activation(
                        out_sb[64:128, jsl], out_ps[64:128, jsl], IDENT,
                        bias=b4_sb[64:128, :],
                    )
                else:
                    nc.vector.tensor_scalar(
                        out_sb[64:128, jsl], out_ps[64:128, jsl],
                        b4_sb[64:128, :], None, ADD,
                    )
                nc.sync.dma_start(outd[64:128, jsl], out_sb[64:128, jsl])

    nc.compile()
    return nc


def _get_compiled():
    if "nc" not in _CACHE:
        _CACHE["nc"] = _build_bass()
    return _CACHE["nc"]


def _prep_in_maps(x, y, W1, b1, W2, b2, W3, b3, W4, b4):
    d = x.shape[1]
    W1a = W1[:d]
    W1b = W1[d:]
    W3P, W4P = _build_packed_weights(W3, W4)

    finpack = np.empty((D, 4), np.float32)
    finpack[:, F_B1] = b1
    finpack[:, F_B2] = np.concatenate([b2, b2])
    finpack[:, F_B3] = np.tile(b3, 16)
    finpack[:, F_B4] = b4[0]

    bin2p = np.empty((D, B_TOT), BF16)
    bin2p[:, B_W1B : B_W1B + D] = W1b.astype(BF16)
    bin2p[:, B_YT : B_YT + J] = y.T.astype(BF16)

    bin3p = np.empty((D, C_TOT), BF16)
    bin3p[:, C_W2 : C_W2 + 64] = W2.astype(BF16)
    bin3p[:, C_W3P : C_W3P + 256] = (
        W3P.transpose(1, 0, 2).reshape(D, 256).astype(BF16)
    )
    bin3p[:, C_W4P : C_W4P + 256] = (
        W4P.transpose(1, 0, 2).reshape(D, 256).astype(BF16)
    )

    W1a_bf = W1a.astype(BF16)
    in_maps = []
    for c in range(N_CORES):
        bin1p = np.empty((D, A_TOT), BF16)
        bin1p[:, A_W1A : A_W1A + D] = W1a_bf
        bin1p[:, A_XT : A_XT + NI] = x[c * NI : (c + 1) * NI].T.astype(BF16)
        in_maps.append(
            {"bin1": bin1p, "bin2": bin2p, "bin3": bin3p, "fin": finpack}
        )
    return in_maps


def run(x, y, W1, b1, W2, b2, W3, b3, W4, b4, **spmd_kwargs):
    """Run the kernel, returning (output, BassKernelResults)."""
    args = [np.asarray(a, np.float32) for a in
            (x, y, W1, b1, W2, b2, W3, b3, W4, b4)]
    in_maps = _prep_in_maps(*args)
    nc = _get_compiled()
    res = run_bass_kernel_spmd(nc, in_maps, list(range(N_CORES)), **spmd_kwargs)
    out = np.concatenate([np.asarray(r["out"]) for r in res.results], axis=0)
    return out.astype(np.float32), res


def kernel(x, y, W1, b1, W2, b2, W3, b3, W4, b4):
    out, _ = run(x, y, W1, b1, W2, b2, W3, b3, W4, b4)
    return out


# revision 23
# speedup vs baseline: 9.8363x; 1.0039x over previous
"""ConcatCritic MLP on 8 Trainium2 NeuronCores.

Computes out[a, b] = f(concat(x[a], y[b])) for a tiny 4-layer MLP
(256->128->64->8->1 with ReLU), i.e. a [1024, 1024] score matrix.

Sharding (per spec hint): x's batch dim across the 8 cores (128 rows each);
y and the weights replicated. Each core computes a [128, 1024] output block.

Dataflow per core (feature-on-partition layout):
  - Split layer 1: concat(x,y) @ W1 = x @ W1[:128] + y @ W1[128:].
    xab[f, i] = (W1a^T @ x^T)[f, i] + b1[f]   (128 x 128, fp32)
    ybT[f, j] = (W1b^T @ y^T)[f, j]           (128 x 1024, bf16)
  - h1_i = relu(ybT + xab[:, i]) -- one tensor_scalar per i (bf16); even i
    on the DVE, odd i on GpSimd (SBUF-only op, so GpSimd is legal and it
    offloads the DVE, which also handles PSUM-side evacuation spill).
  - L2: PE matmul, stationary W2 [128, 64]; even i -> PSUM rows 0:64
    (tile_position (0,0)), odd i -> rows 64:128 ((0,64)); one PSUM tile
    holds a pair of i's -> relu(+b2) evacuates [128, 1024] at once
    (ScalarE, with every 4th pair's second half on the DVE).
  - L3: stationary [128, 64] zero-padded 16-col strips (variant v for pair
    p = 4t+v) accumulate 4 pairs into each 64-row PSUM half; 8 pairs fill a
    dense [128, 1024] "h3pack" (16 i's) -> relu(+b3) evacuates split
    across ScalarE/DVE halves.
  - L4: stationary [128, 64] with one W4 entry per (row-block, i) strip;
    8 groups accumulate into one [128, 1024] PSUM = the core's full output
    block (+b4 on evacuation).

Pipelining: the PE is the roofline engine (~74us of matmul exec), so its
queue must never wait on an evacuation. Both L3 and L4 are DEFERRED: the
PE program order per pair-slot is [L2(p) x4][L4(g-1) at p==2][L3(p-1) x2],
so every matmul's input evacuation completed during the previous slot.
h1 production runs LOOKAHEAD pairs ahead.

All matmuls are bf16 (2 cols/cycle) in 128x64 column-tiling mode
(tile_positions (0,0)/(0,64) only) so the PE never mode-switches. PSUM
accumulation is fp32. x^T/y^T are cast to bf16 on the HOST and arrive in
dependency-ordered DMA chunks so layer 1 starts ~0.2us after launch.
PSUM budget: ps2 2x[128,1024] (4 banks) + ps3 1x (2) + pso 1x (2) = 8.
"""

import numpy as np
import ml_dtypes

import concourse.bass as bass
import concourse.bacc as bacc
import concourse.mybir as mybir
import concourse.tile as tile
from concourse.bass_utils import run_bass_kernel_spmd

BF16 = ml_dtypes.bfloat16
N_CORES = 8
B = 1024
D = 128
NI = B // N_CORES  # 128 rows of x per core
J = B              # full y batch per core
JC = 512           # matmul free-dim chunk (one PSUM bank)

# bin1 [128, 256] bf16: W1A | xT shard     (critical path: layer-1 x side)
A_W1A = 0
A_XT = A_W1A + D
A_TOT = A_XT + NI
# bin2 [128, 1152] bf16: W1B | yT          (layer-1 y side)
B_W1B = 0
B_YT = B_W1B + D
B_TOT = B_YT + J
# bin3 [128, 576] bf16: W2 | W3P | W4P     (needed from first L2 on)
C_W2 = 0
C_W3P = C_W2 + 64
C_W4P = C_W3P + 256
C_TOT = C_W4P + 256
# fin [128, 4] fp32: b1 | b2(x2) | b3(x16) | b4
F_B1, F_B2, F_B3, F_B4 = 0, 1, 2, 3

# h2 evacuation: pairs with (gp % EVAC_DVE_MOD) == EVAC_DVE_PHASE get their
# second column half evacuated by the DVE instead of ScalarE, keeping the
# ScalarE (whose steady load ~= the PE's) from pacing the kernel.
EVAC_DVE_MOD = 1000  # disabled: DVE is loaded with h1 + h3 halves
EVAC_DVE_PHASE = 999

LOOKAHEAD = 3
NPAIR = 64

_CACHE = {}


def _i_local_of_row(r):
    # h3pack row r -> which of the group's 16 i's it holds
    t, v, b = r // 64, (r % 64) // 16, (r % 16) // 8
    return 2 * (4 * t + v) + b


def _build_packed_weights(W3, W4):
    W3P = np.zeros((4, 128, 64), np.float32)
    for v in range(4):
        W3P[v, 0:64, 16 * v : 16 * v + 8] = W3
        W3P[v, 64:128, 16 * v + 8 : 16 * v + 16] = W3
    W4P = np.zeros((4, 128, 64), np.float32)
    for v4 in range(4):
        for r in range(128):
            c = 16 * v4 + _i_local_of_row(r)
            W4P[v4, r, c] = W4[r % 8, 0]
    return W3P, W4P


def _build_bass():
    nc = bacc.Bacc("TRN2", target_bir_lowering=False)
    f32 = mybir.dt.float32
    bf16 = mybir.dt.bfloat16

    bin1 = nc.dram_tensor("bin1", [D, A_TOT], bf16, kind="ExternalInput")
    bin2 = nc.dram_tensor("bin2", [D, B_TOT], bf16, kind="ExternalInput")
    bin3 = nc.dram_tensor("bin3", [D, C_TOT], bf16, kind="ExternalInput")
    fin_d = nc.dram_tensor("fin", [D, 4], f32, kind="ExternalInput")
    outd = nc.dram_tensor("out", [NI, J], f32, kind="ExternalOutput")

    RELU = mybir.ActivationFunctionType.Relu
    IDENT = mybir.ActivationFunctionType.Identity
    ADD = mybir.AluOpType.add
    MAX = mybir.AluOpType.max

    with tile.TileContext(nc) as tc:
        with (
            tc.tile_pool(name="const", bufs=1) as cpool,
            tc.tile_pool(name="work", bufs=14) as work,
            tc.tile_pool(name="h2p", bufs=6) as h2pool,
            tc.tile_pool(name="h3p", bufs=2) as h3pool,
            tc.tile_pool(name="ps2", bufs=2, space="PSUM") as ps2,
            tc.tile_pool(name="ps3", bufs=1, space="PSUM") as ps3,
            tc.tile_pool(name="pso", bufs=1, space="PSUM") as pso,
        ):
            hin1 = cpool.tile([D, A_TOT], bf16)
            hin2 = cpool.tile([D, B_TOT], bf16)
            hin3 = cpool.tile([D, C_TOT], bf16)
            fin = cpool.tile([D, 4], f32)
            xab = cpool.tile([D, NI], f32)
            ybT = cpool.tile([D, J], bf16)
            scratch = cpool.tile([D, 1], f32)

            # dependency-ordered input DMAs on BOTH hardware DMA queues
            # (Sync + ScalarE are the TRN2 HWDGE engines) so the x-side and
            # y-side layer-1 inputs land concurrently. hin2 is split so the
            # W1B + first yT column half (all the first ybT matmuls need)
            # completes one DMA-latency earlier.
            H2SPLIT = B_YT + JC
            nc.sync.dma_start(hin1[:], bin1[:])
            nc.scalar.dma_start(hin2[:, 0:H2SPLIT], bin2[:, 0:H2SPLIT])
            nc.sync.dma_start(fin[:], fin_d[:])
            nc.scalar.dma_start(hin2[:, H2SPLIT:B_TOT], bin2[:, H2SPLIT:B_TOT])
            nc.scalar.dma_start(hin3[:], bin3[:])

            # dummy activation: pulls the ~1.3us ACT_TABLE_LOAD off the
            # critical path (runs while the input DMAs are in flight)
            nc.vector.memset(scratch[:], 0.0)
            nc.scalar.activation(scratch[:], scratch[:], RELU)

            W2_sb = hin3[:, C_W2 : C_W2 + 64]
            b1_sb = fin[:, F_B1 : F_B1 + 1]
            b2_sb = fin[:, F_B2 : F_B2 + 1]
            b3_sb = fin[:, F_B3 : F_B3 + 1]
            b4_sb = fin[:, F_B4 : F_B4 + 1]

            # ---- precompute: xab = W1a^T @ xT + b1 ; ybT = W1b^T @ yT ----
            # xa_ps borrows the pso pool (out_ps is its next generation;
            # its first write happens long after xab is evacuated)
            xa_ps = pso.tile([D, J], mybir.dt.float32, tag="pso")
            for c in range(2):
                nc.tensor.matmul(
                    xa_ps[64 * c : 64 * c + 64, :NI],
                    hin1[:, A_W1A + 64 * c : A_W1A + 64 * c + 64],
                    hin1[:, A_XT : A_XT + NI],
                    tile_position=(0, 64 * c),
                )
            nc.scalar.activation(xab[:], xa_ps[:, :NI], IDENT, bias=b1_sb)

            yb_ps = ps2.tile([D, J], mybir.dt.float32, tag="ps2")
            for jc in range(2):
                for c in range(2):
                    nc.tensor.matmul(
                        yb_ps[64 * c : 64 * c + 64, JC * jc : JC * jc + JC],
                        hin2[:, B_W1B + 64 * c : B_W1B + 64 * c + 64],
                        hin2[:, B_YT + JC * jc : B_YT + JC * jc + JC],
                        tile_position=(0, 64 * c),
                    )
            # evacuate ybT in column halves on both engines so the first
            # h1 tensor_scalars (which are also emitted per-half) start as
            # soon as half the columns are ready
            nc.scalar.activation(ybT[:, 0:JC], yb_ps[:, 0:JC], IDENT)
            nc.vector.tensor_scalar(
                ybT[:, JC:J], yb_ps[:, JC:J], 0.0, None, ADD
            )

            # ---- main loop (software-pipelined emission) ----
            out_ps = pso.tile([D, J], mybir.dt.float32, tag="pso")
            out_sb = cpool.tile([NI, J], f32)

            def x_col(gp):
                g, p = gp // 8, gp % 8
                return 64 * (g // 4) + 16 * (g % 4) + 2 * p

            def emit_h1(gp, half=None):
                h1e = work.tile([D, J], bf16, tag="h1")
                h1o = work.tile([D, J], bf16, tag="h1")
                ie = x_col(gp)
                sl = slice(0, J) if half is None else slice(JC * half, JC * half + JC)
                nc.vector.tensor_scalar(
                    h1e[:, sl], ybT[:, sl], xab[:, ie : ie + 1], 0.0, ADD, MAX
                )
                nc.vector.tensor_scalar(
                    h1o[:, sl], ybT[:, sl], xab[:, ie + 1 : ie + 2], 0.0, ADD, MAX
                )
                return h1e, h1o

            def emit_h1_half(gp, h1eo, half):
                h1e, h1o = h1eo
                ie = x_col(gp)
                sl = slice(JC * half, JC * half + JC)
                nc.vector.tensor_scalar(
                    h1e[:, sl], ybT[:, sl], xab[:, ie : ie + 1], 0.0, ADD, MAX
                )
                nc.vector.tensor_scalar(
                    h1o[:, sl], ybT[:, sl], xab[:, ie + 1 : ie + 2], 0.0, ADD, MAX
                )

            def _emit_l4(g, h3pack):
                t4, v4 = g // 4, g % 4
                for jc in range(2):
                    jsl = slice(JC * jc, JC * jc + JC)
                    nc.tensor.matmul(
                        out_ps[64 * t4 : 64 * t4 + 64, jsl],
                        hin3[:, C_W4P + 64 * v4 : C_W4P + 64 * v4 + 64],
                        h3pack[:, jsl],
                        tile_position=(0, 64 * t4),
                        start=(v4 == 0),
                        stop=(v4 == 3),
                    )

            def _emit_l3(t, v, h2pack, ps3_ref):
                for jc in range(2):
                    jsl = slice(JC * jc, JC * jc + JC)
                    nc.tensor.matmul(
                        ps3_ref[64 * t : 64 * t + 64, jsl],
                        hin3[:, C_W3P + 64 * v : C_W3P + 64 * v + 64],
                        h2pack[:, jsl],
                        tile_position=(0, 64 * t),
                        start=(v == 0),
                        stop=(v == 3),
                    )

            def _emit_h3_evac(ps3_ref):
                # split halves: ScalarE + DVE concurrently, so ps3 (single
                # buffered) frees within one slot
                h3pack = h3pool.tile([D, J], bf16, tag="h3")
                nc.scalar.activation(
                    h3pack[:, 0:JC], ps3_ref[:, 0:JC], RELU, bias=b3_sb
                )
                nc.vector.tensor_scalar(
                    h3pack[:, JC:J], ps3_ref[:, JC:J], b3_sb, 0.0, ADD, MAX
                )
                return h3pack

            # prefill: first two pairs emitted half-by-half (jc0 halves
            # first) so the first L2A matmuls start as soon as the jc0
            # column half of ybT is evacuated
            h1q = {0: emit_h1(0, half=0), 1: emit_h1(1, half=0)}
            emit_h1_half(0, h1q[0], 1)
            emit_h1_half(1, h1q[1], 1)
            h1q[2] = emit_h1(2)
            pend_l4 = None   # (g, h3pack) awaiting L4 emission
            pend_l3 = []     # [(target_gp, t, v, h2pack, ps3_ref, g_done)]
            pend_h3a = None  # ScalarE h3 half, emitted after this slot's h2

            def slot(gp, ps3_t):
                g, p = gp // 8, gp % 8
                nonlocal pend_l4, pend_h3a
                t, v = p // 4, p % 4
                h1e, h1o = h1q.pop(gp)
                ps2_t = ps2.tile([D, J], mybir.dt.float32, tag="ps2")
                for jc in range(2):
                    jsl = slice(JC * jc, JC * jc + JC)
                    nc.tensor.matmul(
                        ps2_t[0:64, jsl], W2_sb, h1e[:, jsl],
                        tile_position=(0, 0),
                    )
                    nc.tensor.matmul(
                        ps2_t[64:128, jsl], W2_sb, h1o[:, jsl],
                        tile_position=(0, 64),
                    )
                # deferred L3s whose target slot arrived (defer 2 for pairs
                # p0..p5, defer 1 for p6/p7 so a group's ps3 completes two
                # slots before the next group's first L3 reuses the banks)
                while pend_l3 and pend_l3[0][0] <= gp:
                    _, pt, pv, ph2, pps3, pg_done = pend_l3.pop(0)
                    _emit_l3(pt, pv, ph2, pps3)
                    if pg_done is not None:
                        # h3 evac: DVE half now (leads the DVE queue);
                        # ScalarE half after this slot's h2 evacuation
                        h3pack = h3pool.tile([D, J], bf16, tag="h3")
                        nc.vector.tensor_scalar(
                            h3pack[:, JC:J], pps3[:, JC:J], b3_sb, 0.0,
                            ADD, MAX,
                        )
                        pend_h3a = (pps3, h3pack)
                        pend_l4 = (pg_done, h3pack)
                h2pack_cur = h2pool.tile([D, J], bf16, tag="h2")
                pend_l3.append(
                    (gp + (1 if p >= 6 else 2), t, v, h2pack_cur, ps3_t,
                     g if p == 7 else None)
                )
                if pend_l4 is not None and p == 5:
                    _emit_l4(*pend_l4)
                    pend_l4 = None
                if g == 4 and p in (6, 7):
                    # rows 0:64 are final (L4 of g3 ran at p==5): evacuate
                    # one column half per slot on the DVE (+b4), then DMA
                    jsl = slice(JC * (p - 6), JC * (p - 6) + JC)
                    nc.vector.tensor_scalar(
                        out_sb[0:64, jsl], out_ps[0:64, jsl],
                        b4_sb[0:64, :], None, ADD,
                    )
                    nc.sync.dma_start(outd[0:64, jsl], out_sb[0:64, jsl])
                # one h2 spill per group (pair p1): its jc1 half goes to
                # the DVE, giving ScalarE slack to absorb the h3 burst
                spill = p == 1
                if spill:
                    nc.vector.tensor_scalar(
                        h2pack_cur[:, JC:J], ps2_t[:, JC:J], b2_sb, 0.0,
                        ADD, MAX,
                    )
                if gp + LOOKAHEAD < NPAIR:
                    h1q[gp + LOOKAHEAD] = emit_h1(gp + LOOKAHEAD)
                if spill:
                    nc.scalar.activation(
                        h2pack_cur[:, 0:JC], ps2_t[:, 0:JC], RELU, bias=b2_sb
                    )
                else:
                    nc.scalar.activation(
                        h2pack_cur[:], ps2_t[:], RELU, bias=b2_sb
                    )
                if pend_h3a is not None:
                    pps3, h3pack = pend_h3a
                    nc.scalar.activation(
                        h3pack[:, 0:JC], pps3[:, 0:JC], RELU, bias=b3_sb
                    )
                    pend_h3a = None

            for g in range(8):
                ps3_t = ps3.tile([D, J], mybir.dt.float32, tag="ps3")
                for p in range(8):
                    slot(g * 8 + p, ps3_t)

            # epilogue: drain deferred L3s (p6, p7), h3 evac, final L4,
            # then DMA rows 64:128 straight from PSUM per column half
            while pend_l3:
                _, pt, pv, ph2, pps3, pg_done = pend_l3.pop(0)
                _emit_l3(pt, pv, ph2, pps3)
            h3pack = h3pool.tile([D, J], bf16, tag="h3")
            nc.scalar.activation(
                h3pack[:, 0:JC], pps3[:, 0:JC], RELU, bias=b3_sb
            )
            nc.vector.tensor_scalar(
                h3pack[:, JC:J], pps3[:, JC:J], b3_sb, 0.0, ADD, MAX
            )
            t4, v4 = pg_done // 4, pg_done % 4
            for jc in range(2):
                jsl = slice(JC * jc, JC * jc + JC)
                nc.tensor.matmul(
                    out_ps[64 * t4 : 64 * t4 + 64, jsl],
                    hin3[:, C_W4P + 64 * v4 : C_W4P + 64 * v4 + 64],
                    h3pack[:, jsl],
                    tile_position=(0, 64 * t4),
                    start=(v4 == 0),
                    stop=(v4 == 3),
                )
                if jc == 0:
                    nc.scalar.activation(
                        out_sb[64:128, jsl], out_ps[64:128, jsl], IDENT,
                        bias=b4_sb[64:128, :],
                    )
                    nc.sync.dma_start(outd[64:128, jsl], out_sb[64:128, jsl])
                else:
                    nc.vector.tensor_scalar(
                        out_sb[64:128, jsl], out_ps[64:128, jsl],
                        b4_sb[64:128, :], None, ADD,
                    )
                    nc.scalar.dma_start(outd[64:128, jsl], out_sb[64:128, jsl])

    nc.compile()
    return nc


def _get_compiled():
    if "nc" not in _CACHE:
        _CACHE["nc"] = _build_bass()
    return _CACHE["nc"]


def _prep_in_maps(x, y, W1, b1, W2, b2, W3, b3, W4, b4):
    d = x.shape[1]
    W1a = W1[:d]
    W1b = W1[d:]
    W3P, W4P = _build_packed_weights(W3, W4)

    finpack = np.empty((D, 4), np.float32)
    finpack[:, F_B1] = b1
    finpack[:, F_B2] = np.concatenate([b2, b2])
    finpack[:, F_B3] = np.tile(b3, 16)
    finpack[:, F_B4] = b4[0]

    bin2p = np.empty((D, B_TOT), BF16)
    bin2p[:, B_W1B : B_W1B + D] = W1b.astype(BF16)
    bin2p[:, B_YT : B_YT + J] = y.T.astype(BF16)

    bin3p = np.empty((D, C_TOT), BF16)
    bin3p[:, C_W2 : C_W2 + 64] = W2.astype(BF16)
    bin3p[:, C_W3P : C_W3P + 256] = (
        W3P.transpose(1, 0, 2).reshape(D, 256).astype(BF16)
    )
    bin3p[:, C_W4P : C_W4P + 256] = (
        W4P.transpose(1, 0, 2).reshape(D, 256).astype(BF16)
    )

    W1a_bf = W1a.astype(BF16)
    in_maps = []
    for c in range(N_CORES):
        bin1p = np.empty((D, A_TOT), BF16)
        bin1p[:, A_W1A : A_W1A + D] = W1a_bf
        bin1p[:, A_XT : A_XT + NI] = x[c * NI : (c + 1) * NI].T.astype(BF16)
        in_maps.append(
            {"bin1": bin1p, "bin2": bin2p, "bin3": bin3p, "fin": finpack}
        )
    return in_maps


def run(x, y, W1, b1, W2, b2, W3, b3, W4, b4, **spmd_kwargs):
    """Run the kernel, returning (output, BassKernelResults)."""
    args = [np.asarray(a, np.float32) for a in
            (x, y, W1, b1, W2, b2, W3, b3, W4, b4)]
    in_maps = _prep_in_maps(*args)
    nc = _get_compiled()
    res = run_bass_kernel_spmd(nc, in_maps, list(range(N_CORES)), **spmd_kwargs)
    out = np.concatenate([np.asarray(r["out"]) for r in res.results], axis=0)
    return out.astype(np.float32), res


def kernel(x, y, W1, b1, W2, b2, W3, b3, W4, b4):
    out, _ = run(x, y, W1, b1, W2, b2, W3, b3, W4, b4)
    return out
